# revision 12
# baseline (speedup 1.0000x reference)
"""Multi-head attention forward on 8 Trainium2 NeuronCores.

Problem: B=4, S=2048, E=1024, H=16, D=64 (fp32 in/out).

Sharding: 8 cores = (batch b, sequence half). Each core handles the full
key/value sequence of its batch (K/V projections computed redundantly by the
2 cores sharing a batch) and 1024 query rows, so outputs are disjoint and no
collective is needed. Inputs are host-rolled so each core's query rows are
rows 0:1024 of its x — softmax over keys is permutation invariant, so rolling
the key axis does not change the result.

On-core pipeline (all layouts chosen so no on-chip tensor ever needs a
transpose except x itself):
  x -> x^T (PE transpose, fp32)           [e, s]
  K^T = Wk^T x^T (fp32r), Q^T likewise    [n, s]  (cast bf16)
  V   = x Wv (fp32r)                      [s, n]  (cast bf16, +ones column)
  scores^T[k,q] = K_h^T.T @ Q_h^T (bf16)  per head, PSUM
  attn = exp(scores/8) (ACT, no max-sub — scores are N(0,1), exp<=~500)
  ctx^T[d,q]/denom[q] = [V_h|1].T @ attn^T  (ones column gives denom free)
  y = ctx^T.T @ Wo (bf16) -> fp32 out
"""

import os
import sys
import types

import numpy as np

sys.path.insert(0, "/opt/trn_rl_repo")

B, S, E, H = 4, 2048, 1024, 16
D = E // H          # 64
Q = S // 2          # query rows per core
NCORES = 8

_compiled = None


def _install_prof_hook():
    try:
        import antenv.axon_hooks  # noqa: F401
        return
    except ImportError:
        pass
    try:
        import antenv
        from trn_agent_boot.trn_boot import _ntff_profile_via_ctypes
    except ImportError:
        return
    mod = types.ModuleType("antenv.axon_hooks")
    mod._hook = None
    mod.set_axon_ntff_profile_hook = lambda h: setattr(mod, "_hook", h)
    mod.get_axon_ntff_profile_hook = lambda: mod._hook
    sys.modules["antenv.axon_hooks"] = mod
    antenv.axon_hooks = mod
    try:
        mod._hook = _ntff_profile_via_ctypes("/opt/axon/libaxon_pjrt.so")
    except Exception:
        mod._hook = None


def _build():
    import concourse.bass as bass
    from concourse import bacc
    import concourse.mybir as mybir
    import concourse.tile as tile_mod
    from concourse import tile_utils
    from concourse.tile import TileContext
    from concourse.masks import make_identity

    # Default cap (192KB/partition) leaves usable SBUF on the table; we need
    # ~200KB/partition. Cayman has 208KB usable.
    tile_utils.max_sbuf_usage = 204 * 1024

    F32 = mybir.dt.float32
    F32R = mybir.dt.float32r
    BF16 = mybir.dt.bfloat16
    Exp = mybir.ActivationFunctionType.Exp

    nc = bacc.Bacc("TRN2", target_bir_lowering=False, debug=False)

    xb = nc.dram_tensor("xb", [S, E], F32, kind="ExternalInput")
    wq = nc.dram_tensor("wq", [E, E], F32R, kind="ExternalInput")
    wk = nc.dram_tensor("wk", [E, E], F32R, kind="ExternalInput")
    wv = nc.dram_tensor("wv", [E, E], F32R, kind="ExternalInput")
    wo = nc.dram_tensor("wo", [E, E], F32, kind="ExternalInput")
    y = nc.dram_tensor("y", [Q, E], F32, kind="ExternalOutput")

    # views with 128-partition chunking of the contraction axis
    wq_v = wq.ap().rearrange("(eb p) n -> p eb n", p=128)   # [128, 8, 1024]
    wk_v = wk.ap().rearrange("(eb p) n -> p eb n", p=128)
    wv_v = wv.ap().rearrange("(eb p) n -> p eb n", p=128)
    wo_v = wo.ap().rearrange("(eb p) n -> p eb n", p=128)
    x_v = xb.ap().rearrange("(sb p) e -> sb p e", p=128)    # [16, 128, 1024]
    y_v = y.ap().rearrange("(sb p) e -> sb p e", p=128)     # [8, 128, 1024]

    EB = E // 128        # 8 e-chunks
    SB = S // 128        # 16 s blocks
    QB = Q // 128        # 8 query blocks
    SC = 4               # s-chunks of 512 for x^T staging
    KB = S // 128        # 16 key blocks

    from contextlib import ExitStack
    with TileContext(nc) as tc:
        with ExitStack() as es:
            constp = es.enter_context(tc.tile_pool(name="const", bufs=1))
            kTp = es.enter_context(tc.tile_pool(name="kT", bufs=1))
            qTp = es.enter_context(tc.tile_pool(name="qT", bufs=1))
            vp = es.enter_context(tc.tile_pool(name="vA", bufs=1))
            ctxp = es.enter_context(tc.tile_pool(name="ctx", bufs=1))
            attnp = es.enter_context(tc.tile_pool(name="attn", bufs=1))
            xTp = es.enter_context(tc.tile_pool(name="xT", bufs=1))
            xsp = es.enter_context(tc.tile_pool(name="xs", bufs=2))
            wkqp = es.enter_context(tc.tile_pool(name="wkq", bufs=4))
            wvp = es.enter_context(tc.tile_pool(name="wvp", bufs=2))
            wobp = es.enter_context(tc.tile_pool(name="wob", bufs=2))
            wofp = es.enter_context(tc.tile_pool(name="wof", bufs=2))
            ytp = es.enter_context(tc.tile_pool(name="yt", bufs=2))
            nrmp = es.enter_context(tc.tile_pool(name="nrm", bufs=2))
            stgp = es.enter_context(tc.tile_pool(name="stg", bufs=1))
            psA = es.enter_context(tc.tile_pool(name="psA", bufs=2, space="PSUM"))
            psB = es.enter_context(tc.tile_pool(name="psB", bufs=4, space="PSUM"))
            ident = constp.tile([128, 128], F32)
            make_identity(nc, ident[:])

            kT = kTp.tile([128, EB, S], BF16)        # K^T  [n, s]
            qT = qTp.tile([128, EB, Q], BF16)        # Q^T  [n, q]
            vA = vp.tile([128, SB, H, D + 1], BF16)  # V (+ones col) [s, h, d|1]
            ctx = ctxp.tile([128, EB, Q], BF16)      # ctx^T [e, q]
            # per-head attn buffers for one 512-wide q-chunk, 2 sub-tiles each
            # (sub-tiling narrows WAR serialization between blocks)
            attn_t = [[attnp.tile([128, 8, 512], BF16, tag=f"attn{h}{i}",
                                  name=f"attn{h}{i}")
                       for i in range(2)] for h in range(2)]

            nc.gpsimd.memset(vA[:, :, :, D], 1.0)    # ones column

            # ---------------- projections, per s-chunk of 512 ----------------
            for sc in range(SC):
                xt = xTp.tile([128, EB, 512], F32R)  # x^T chunk [e, 512 s]
                for si in range(4):                  # 4 s-blocks per chunk
                    sb = sc * 4 + si
                    xs = xsp.tile([128, E], F32)
                    nc.sync.dma_start(xs[:], x_v[sb])
                    for eb in range(EB):
                        tp = psB.tile([128, 128], F32, tag="b")
                        nc.tensor.transpose(tp[:], xs[:, eb * 128:(eb + 1) * 128], ident[:])
                        nc.scalar.copy(xt[:, eb, si * 128:(si + 1) * 128], tp[:])

                # K^T (and Q^T for sc<2): accumulate over e-chunks
                for proj, wdram, dst in (("k", wk_v, kT), ("q", wq_v, qT)):
                    if proj == "q" and sc >= 2:
                        continue
                    for nb in range(EB):
                        ps = psB.tile([128, 512], F32, tag="b")
                        for eb in range(EB):
                            wt = wkqp.tile([128, 128], F32R, tag="wkq")
                            nc.sync.dma_start(wt[:], wdram[:, eb, nb * 128:(nb + 1) * 128])
                            nc.tensor.matmul(ps[:], wt[:], xt[:, eb, :],
                                             start=(eb == 0), stop=(eb == EB - 1))
                        nc.vector.tensor_copy(dst[:, nb, sc * 512:(sc + 1) * 512], ps[:])

                # V natural layout: [s_blk, n] = x^T.T @ Wv
                for nc2 in range(2):
                    wvt = [None, None]
                    for ebh in range(2):
                        wvh = wvp.tile([128, 4, 512], F32R, tag="wv", name=f"wv{sc}_{nc2}_{ebh}")
                        nc.sync.dma_start(
                            wvh[:], wv_v[:, ebh * 4:(ebh + 1) * 4,
                                         nc2 * 512:(nc2 + 1) * 512])
                        wvt[ebh] = wvh
                    for si in range(4):
                        sb = sc * 4 + si
                        ps = psB.tile([128, 512], F32, tag="b")
                        for eb in range(EB):
                            nc.tensor.matmul(ps[:], xt[:, eb, si * 128:(si + 1) * 128],
                                             wvt[eb // 4][:, eb % 4, :],
                                             start=(eb == 0), stop=(eb == EB - 1))
                        nc.vector.tensor_copy(
                            vA[:, sb, nc2 * 8:(nc2 + 1) * 8, 0:D],
                            ps[:].rearrange("p (h d) -> p h d", d=D))

            # ---------------- attention, per head pair ----------------
            inv_sqrt_d = 1.0 / float(np.sqrt(D))
            for j in range(H // 2):
                for qc in range(2):
                    qs = slice(qc * 512, (qc + 1) * 512)
                    # scores + exp: psum tile holds 2 k-blocks for this q-chunk;
                    # the two heads' matmuls alternate row groups 0/64 so the
                    # PE overlaps them (row tiling)
                    for kbp in range(KB // 2):
                        sps = [psA.tile([128, 1024], F32, tag="sc",
                                        name=f"sc{j}_{qc}_{kbp}_{s}")
                               for s in range(2)]
                        for ki in range(2):
                            kb = 2 * kbp + ki
                            for hh in range(2):
                                p0 = hh * 64
                                nc.tensor.matmul(
                                    sps[hh][:, ki * 512:(ki + 1) * 512],
                                    kT[p0:p0 + 64, j, kb * 128:(kb + 1) * 128],
                                    qT[p0:p0 + 64, j, qs],
                                    start=True, stop=True)
                        for hh in range(2):
                            nc.scalar.activation(
                                attn_t[hh][kbp // 4][:, (kbp % 4) * 2:(kbp % 4) * 2 + 2, :]
                                .rearrange("p a b -> p (a b)"),
                                sps[hh][:], Exp, scale=inv_sqrt_d)

                    # ctx^T (+denominator row) per head, then normalize
                    for hh in range(2):
                        h = 2 * j + hh
                        cps = psB.tile([128, 512], F32, tag="b")
                        for kb in range(KB):
                            nc.tensor.matmul(
                                cps[0:D + 1, :],
                                vA[:, kb, h, :],
                                attn_t[hh][kb // 8][:, kb % 8, :],
                                start=(kb == 0), stop=(kb == KB - 1))
                        den = nrmp.tile([1, 512], F32, tag="den")
                        nc.vector.tensor_copy(den[:], cps[D:D + 1, :])
                        nc.vector.reciprocal(den[:], den[:])
                        bcast = nrmp.tile([64, 512], F32, tag="bc")
                        nc.gpsimd.partition_broadcast(bcast[:], den[:])
                        if hh == 0:
                            nc.vector.tensor_mul(
                                ctx[0:64, j, qs], cps[0:D, :], bcast[:])
                        else:
                            stg = stgp.tile([64, 512], BF16)
                            nc.vector.tensor_mul(stg[:], cps[0:D, :], bcast[:])
                            nc.sync.dma_start(ctx[64:128, j, qs], stg[:])

            # ---------------- output projection ----------------
            for nc2 in range(2):
                for sbh in range(2):
                    pss = [psB.tile([128, 512], F32, tag="b",
                                    name=f"yps{nc2}_{sbh}_{i}") for i in range(4)]
                    for eb in range(EB):
                        wof = wofp.tile([128, 512], F32)
                        nc.sync.dma_start(wof[:],
                                          wo_v[:, eb, nc2 * 512:(nc2 + 1) * 512])
                        wob = wobp.tile([128, 512], BF16)
                        nc.vector.tensor_copy(wob[:], wof[:])
                        for si in range(4):
                            sb = sbh * 4 + si
                            nc.tensor.matmul(pss[si][:],
                                             ctx[:, eb, sb * 128:(sb + 1) * 128],
                                             wob[:],
                                             start=(eb == 0), stop=(eb == EB - 1))
                    for si in range(4):
                        sb = sbh * 4 + si
                        yt = ytp.tile([128, 512], F32)
                        nc.vector.tensor_copy(yt[:], pss[si][:])
                        nc.sync.dma_start(y_v[sb][:, nc2 * 512:(nc2 + 1) * 512], yt[:])

    nc.compile()
    return nc


def kernel(x, Wq, Wk, Wv, Wo):
    global _compiled
    _install_prof_hook()
    from concourse import bass_utils

    if _compiled is None:
        _compiled = _build()
    nc = _compiled

    x = np.ascontiguousarray(x, dtype=np.float32)
    in_maps = []
    for c in range(NCORES):
        b, half = c // 2, c % 2
        xc = np.roll(x[b], -Q * half, axis=0) if half else x[b]
        in_maps.append({
            "xb": np.ascontiguousarray(xc),
            "wq": Wq.astype(np.float32), "wk": Wk.astype(np.float32),
            "wv": Wv.astype(np.float32), "wo": Wo.astype(np.float32),
        })

    trace = bool(int(os.environ.get("KERNEL_TRACE", "0")))
    res = bass_utils.run_bass_kernel_spmd(
        nc, in_maps, core_ids=list(range(NCORES)), trace=trace)
    kernel.last_result = res

    out = np.empty((B, S, E), dtype=np.float32)
    for c in range(NCORES):
        b, half = c // 2, c % 2
        out[b, half * Q:(half + 1) * Q] = res.results[c]["y"]
    return out


# revision 15
# speedup vs baseline: 1.3682x; 1.3682x over previous
"""Multi-head attention forward on 8 Trainium2 NeuronCores.

Problem: B=4, S=2048, E=1024, H=16, D=64 (fp32 in/out).

Sharding: 8 cores = (batch b, sequence half). Each core handles the full
key/value sequence of its batch (K/V projections computed redundantly by the
2 cores sharing a batch) and 1024 query rows, so outputs are disjoint and no
collective is needed. Inputs are host-rolled so each core's query rows are
rows 0:1024 of its x — softmax over keys is permutation invariant, so rolling
the key axis does not change the result.

All matmuls run in bf16 (inputs host-cast; fp32 PSUM accumulation), which on
TRN2 gets fast-weight-load and LDWEIGHTS/matmul overlap. Layouts are chosen
so nothing on chip ever needs a transpose: x^T arrives via DMA-transpose
(legal for 2-byte dtypes), projections produce K^T/Q^T in [n, s] form
directly, V in natural [s, n] form, and scores are built transposed
([k, q]) so the softmax denominator comes from a ones-column in V and the
attn@V contraction needs no reshuffle.

  x^T (DMA transpose)                     [e, s]
  K^T = Wk^T x^T, Q^T likewise            [n, s]
  V   = x Wv  (+ones col)                 [s, h, d|1]
  scores^T[k,q] = K_h^T.T @ Q_h^T         per head pair (PE row tiling)
  attn = exp(scores/8)  (ACT; no max-subtraction needed: scores ~ N(0,1))
  ctx^T[d,q], denom[q] = [V_h|1].T @ attn^T
  y = (ctx^T/denom).T @ Wo -> fp32
"""

import os
import sys
import types

import numpy as np

sys.path.insert(0, "/opt/trn_rl_repo")

B, S, E, H = 4, 2048, 1024, 16
D = E // H          # 64
Q = S // 2          # query rows per core
NCORES = 8

_compiled = None


def _install_prof_hook():
    try:
        import antenv.axon_hooks  # noqa: F401
        return
    except ImportError:
        pass
    try:
        import antenv
        from trn_agent_boot.trn_boot import _ntff_profile_via_ctypes
    except ImportError:
        return
    mod = types.ModuleType("antenv.axon_hooks")
    mod._hook = None
    mod.set_axon_ntff_profile_hook = lambda h: setattr(mod, "_hook", h)
    mod.get_axon_ntff_profile_hook = lambda: mod._hook
    sys.modules["antenv.axon_hooks"] = mod
    antenv.axon_hooks = mod
    try:
        mod._hook = _ntff_profile_via_ctypes("/opt/axon/libaxon_pjrt.so")
    except Exception:
        mod._hook = None


def _build():
    from contextlib import ExitStack

    from concourse import bacc
    import concourse.mybir as mybir
    from concourse import tile_utils
    from concourse.tile import TileContext

    tile_utils.max_sbuf_usage = 204 * 1024  # default 192K; cayman has 208K usable

    F32 = mybir.dt.float32
    BF16 = mybir.dt.bfloat16
    Exp = mybir.ActivationFunctionType.Exp

    nc = bacc.Bacc("TRN2", target_bir_lowering=False, debug=False)

    xb = nc.dram_tensor("xb", [S, E], BF16, kind="ExternalInput")
    wq = nc.dram_tensor("wq", [E, E], BF16, kind="ExternalInput")
    wk = nc.dram_tensor("wk", [E, E], BF16, kind="ExternalInput")
    wv = nc.dram_tensor("wv", [E, E], BF16, kind="ExternalInput")
    wo = nc.dram_tensor("wo", [E, E], BF16, kind="ExternalInput")
    y = nc.dram_tensor("y", [Q, E], F32, kind="ExternalOutput")

    wq_v = wq.ap().rearrange("(eb p) n -> p eb n", p=128)   # [128, 8, 1024]
    wk_v = wk.ap().rearrange("(eb p) n -> p eb n", p=128)
    wv_v = wv.ap().rearrange("(eb p) n -> p eb n", p=128)
    wo_v = wo.ap().rearrange("(eb p) n -> p eb n", p=128)
    y_v = y.ap().rearrange("(sb p) e -> sb p e", p=128)     # [8, 128, 1024]

    EB = E // 128        # 8 e-chunks
    SB = S // 128        # 16 s blocks
    QB = Q // 128        # 8 query blocks
    SC = 4               # s-chunks of 512 for x^T staging
    KB = S // 128        # 16 key blocks

    with TileContext(nc) as tc:
        with ExitStack() as es:
            kTp = es.enter_context(tc.tile_pool(name="kT", bufs=1))
            qTp = es.enter_context(tc.tile_pool(name="qT", bufs=1))
            vp = es.enter_context(tc.tile_pool(name="vA", bufs=1))
            ctxp = es.enter_context(tc.tile_pool(name="ctx", bufs=1))
            attnp = es.enter_context(tc.tile_pool(name="attn", bufs=1))
            xTp = es.enter_context(tc.tile_pool(name="xT", bufs=2))
            wkqp = es.enter_context(tc.tile_pool(name="wkq", bufs=3))
            wvp = es.enter_context(tc.tile_pool(name="wvp", bufs=2))
            wobp = es.enter_context(tc.tile_pool(name="wob", bufs=2))
            ytp = es.enter_context(tc.tile_pool(name="yt", bufs=2))
            nrmp = es.enter_context(tc.tile_pool(name="nrm", bufs=2))
            stgp = es.enter_context(tc.tile_pool(name="stg", bufs=2))
            psA = es.enter_context(tc.tile_pool(name="psA", bufs=2, space="PSUM"))
            psB = es.enter_context(tc.tile_pool(name="psB", bufs=4, space="PSUM"))

            kT = kTp.tile([128, EB, S], BF16)        # K^T  [n, s]
            qT = qTp.tile([128, EB, Q], BF16)        # Q^T  [n, q]
            # V with a ones column per head: even heads [V|1], odd heads [1|V]
            # (odd heads' ctx lands at PSUM partitions 64:128 so the
            # normalization multiply stays partition-aligned)
            vA = vp.tile([128, SB, H, D + 1], BF16)
            ctx = ctxp.tile([128, EB, Q], BF16)      # ctx^T [e, q]
            attn_t = [[attnp.tile([128, 8, 512], BF16, tag=f"attn{h}{i}",
                                  name=f"attn{h}{i}")
                       for i in range(2)] for h in range(2)]

            nc.gpsimd.memset(vA[:, :, :, D], 1.0)      # ones column (all heads)

            # ---------------- projections, per s-chunk of 512 ----------------
            for sc in range(SC):
                xt = xTp.tile([128, EB, 512], BF16)  # x^T chunk [e, 512 s]
                for eb in range(EB):
                    nc.sync.dma_start_transpose(
                        xt[:, eb, :],
                        xb.ap()[sc * 512:(sc + 1) * 512, eb * 128:(eb + 1) * 128])

                # K^T (and Q^T for sc<2): accumulate over e-chunks
                for proj, wdram, dst in (("k", wk_v, kT), ("q", wq_v, qT)):
                    if proj == "q" and sc >= 2:
                        continue
                    for nb in range(EB):
                        wt = wkqp.tile([128, EB, 128], BF16, tag="wkq")
                        nc.sync.dma_start(wt[:], wdram[:, :, nb * 128:(nb + 1) * 128])
                        ps = psB.tile([128, 512], F32, tag="b")
                        for eb in range(EB):
                            nc.tensor.matmul(ps[:], wt[:, eb, :], xt[:, eb, :],
                                             start=(eb == 0), stop=(eb == EB - 1))
                        nc.vector.tensor_copy(dst[:, nb, sc * 512:(sc + 1) * 512], ps[:])

                # V natural layout: [s_blk, n] = x^T.T @ Wv
                for nc2 in range(2):
                    wvt = [None, None]
                    for ebh in range(2):
                        wvh = wvp.tile([128, 4, 512], BF16, tag="wv",
                                       name=f"wv{sc}_{nc2}_{ebh}")
                        nc.sync.dma_start(
                            wvh[:], wv_v[:, ebh * 4:(ebh + 1) * 4,
                                         nc2 * 512:(nc2 + 1) * 512])
                        wvt[ebh] = wvh
                    for si in range(4):
                        sb = sc * 4 + si
                        ps = psB.tile([128, 512], F32, tag="b")
                        for eb in range(EB):
                            nc.tensor.matmul(ps[:], xt[:, eb, si * 128:(si + 1) * 128],
                                             wvt[eb // 4][:, eb % 4, :],
                                             start=(eb == 0), stop=(eb == EB - 1))
                        nc.vector.tensor_copy(
                            vA[:, sb, nc2 * 8:(nc2 + 1) * 8, 0:D],
                            ps[:].rearrange("p (h d) -> p h d", d=D))

            # ---------------- attention, per (head pair, q-chunk) ----------------
            inv_sqrt_d = 1.0 / float(np.sqrt(D))
            for j in range(H // 2):
                for qc in range(2):
                    qs = slice(qc * 512, (qc + 1) * 512)
                    for kbp in range(KB // 2):
                        sps = [psA.tile([128, 1024], F32, tag="sc",
                                        name=f"sc{j}_{qc}_{kbp}_{s}")
                               for s in range(2)]
                        for ki in range(2):
                            kb = 2 * kbp + ki
                            for hh in range(2):
                                p0 = hh * 64
                                nc.tensor.matmul(
                                    sps[hh][:, ki * 512:(ki + 1) * 512],
                                    kT[p0:p0 + 64, j, kb * 128:(kb + 1) * 128],
                                    qT[p0:p0 + 64, j, qs],
                                    start=True, stop=True)
                        for hh in range(2):
                            nc.scalar.activation(
                                attn_t[hh][kbp // 4][:, (kbp % 4) * 2:(kbp % 4) * 2 + 2, :]
                                .rearrange("p a b -> p (a b)"),
                                sps[hh][:], Exp, scale=inv_sqrt_d)

                    # ctx^T (+denominator row) per head, then normalize
                    for hh in range(2):
                        h = 2 * j + hh
                        cps = psB.tile([128, 512], F32, tag="b")
                        for kb in range(KB):
                            nc.tensor.matmul(
                                cps[0:D + 1, :],
                                vA[:, kb, h, :],
                                attn_t[hh][kb // 8][:, kb % 8, :],
                                start=(kb == 0), stop=(kb == KB - 1))
                        den = nrmp.tile([1, 512], F32, tag="den")
                        nc.vector.tensor_copy(den[:], cps[D:D + 1, :])
                        nc.vector.reciprocal_approx_fast(den[:], den[:])
                        bcast = nrmp.tile([64, 512], F32, tag="bc")
                        nc.gpsimd.partition_broadcast(bcast[:], den[:])
                        if hh == 0:
                            nc.vector.tensor_mul(
                                ctx[0:64, j, qs], cps[0:D, :], bcast[:])
                        else:
                            stg = stgp.tile([64, 512], BF16, tag="stg")
                            nc.vector.tensor_mul(stg[:], cps[0:D, :], bcast[:])
                            nc.sync.dma_start(ctx[64:128, j, qs], stg[:])

            # ---------------- output projection ----------------
            for nc2 in range(2):
                for sbh in range(2):
                    pss = [psB.tile([128, 512], F32, tag="b",
                                    name=f"yps{nc2}_{sbh}_{i}") for i in range(4)]
                    for eb in range(EB):
                        wob = wobp.tile([128, 512], BF16)
                        nc.sync.dma_start(wob[:],
                                          wo_v[:, eb, nc2 * 512:(nc2 + 1) * 512])
                        for si in range(4):
                            sb = sbh * 4 + si
                            nc.tensor.matmul(pss[si][:],
                                             ctx[:, eb, sb * 128:(sb + 1) * 128],
                                             wob[:],
                                             start=(eb == 0), stop=(eb == EB - 1))
                    for si in range(4):
                        sb = sbh * 4 + si
                        yt = ytp.tile([128, 512], F32)
                        nc.vector.tensor_copy(yt[:], pss[si][:])
                        nc.sync.dma_start(y_v[sb][:, nc2 * 512:(nc2 + 1) * 512], yt[:])

    nc.compile()
    return nc


def kernel(x, Wq, Wk, Wv, Wo):
    global _compiled
    _install_prof_hook()
    import ml_dtypes
    from concourse import bass_utils

    if _compiled is None:
        _compiled = _build()
    nc = _compiled

    bf16 = ml_dtypes.bfloat16
    x = np.ascontiguousarray(x, dtype=np.float32)
    wq_b = np.ascontiguousarray(np.asarray(Wq, dtype=np.float32).astype(bf16))
    wk_b = np.ascontiguousarray(np.asarray(Wk, dtype=np.float32).astype(bf16))
    wv_b = np.ascontiguousarray(np.asarray(Wv, dtype=np.float32).astype(bf16))
    wo_b = np.ascontiguousarray(np.asarray(Wo, dtype=np.float32).astype(bf16))

    in_maps = []
    for c in range(NCORES):
        b, half = c // 2, c % 2
        xc = np.roll(x[b], -Q * half, axis=0) if half else x[b]
        in_maps.append({
            "xb": np.ascontiguousarray(xc.astype(bf16)),
            "wq": wq_b, "wk": wk_b, "wv": wv_b, "wo": wo_b,
        })

    trace = bool(int(os.environ.get("KERNEL_TRACE", "0")))
    res = bass_utils.run_bass_kernel_spmd(
        nc, in_maps, core_ids=list(range(NCORES)), trace=trace)
    kernel.last_result = res

    out = np.empty((B, S, E), dtype=np.float32)
    for c in range(NCORES):
        b, half = c // 2, c % 2
        out[b, half * Q:(half + 1) * Q] = res.results[c]["y"]
    return out


kernel.last_result = None
